# revision 33
# baseline (speedup 1.0000x reference)
"""VQ codebook squared-distance kernel for Trainium2 (8 NeuronCores).

Computes dist[n,k,l] = (||x[n,:,l]||^2 + ||w[k,:]||^2 - 2*x[n,:,l].w[k,:]) / scale^2
for x (32,128,3136) f32, weight (64,128) f32, scale (1,) f32 -> out (32,64,3136) f32.

Sharding: data-parallel over N (4 per core); weight/scale replicated.

v3 design notes (per-core; v1 37.5us, v2 38.4us measured):
  - Input stream is HBM-stack-roofline-bound (~343 GB/s/core with both
    NCs of a stack active): 6.42 MB f32 x read in ~17.4us. Not
    improvable; everything else is about the edges of the stream.
  - v1's killer: one SDMA engine (the doc'd engine-7/15 SWDGE
    descriptor-ring port contention) lags the other 15, and the lag
    grows whenever the Q7 ring is being written. The last input
    transfer's completion sem fired 3.8us after its bytes landed,
    gating the whole tail. Mitigation here: coarse transfers (full
    images for n0..n2 -> 128 descriptors each instead of 256+) and
    NO output descriptors on the SWDGE ring at all.
  - Outputs ride HWDGE (nc.sync) instead: no Q7 ring writes, no
    queueing behind the straggler's input backlog, faster dispatch
    (RTL descgen ~0.6us vs Q7 ~1.65us). Output pieces are gated late
    (pair-0 ships as one full-L write when its last epilogue lands
    ~24us) to limit read/write turnaround mixing during the stream.
  - The last image (n3) streams in shrinking pieces (4/2/1/1 chunks)
    so the final dependency chain after the last input byte is just:
    2 matmuls + one 392-col epilogue (split ACT||DVE halves) + one
    50 KB HWDGE write + ~2us completion receipt.
  - Output is offset fp8: e4m3(dist - 2D/s^2), host adds the offset
    back; rel_l2 ~3e-3 vs the 2e-2 budget, half the write traffic.
  - PE: psum = (-2Wt)f16 @ x_f16 + ones_f16 @ (x^2)_f16, two images
    per PSUM tile via column tiling (tile_position (0,0)/(0,64)).
  - NEFF postamble: walrus emits a FIXED 257 per-semaphore clears
    split across the 5 engines (measured invariant to kernel size);
    the pacer is the Tensor sequencer at ~115ns/clear when the HAM
    clock gate has re-throttled (PE idle >3.4us). A dummy matmul
    gated on the final epilogue keeps PE at K=8/8 through most of
    the clear window (~57ns/clear), moving the pacer to Scalar.
  - scale broadcast 1->128 via 1-col fp32 matmul; weight transpose on
    PE (identity built early on gpsimd).
"""

import numpy as np

N, D, L, K = 32, 128, 3136, 64
N_CORES = 8
NS = N // N_CORES          # n's per core
LC = 392                   # matmul chunk (8 per image, one PSUM bank)
HC = 196                   # half-chunk for the split tail epilogues

_cache = {}


def _build():
    import concourse.bacc as bacc
    import concourse.mybir as mybir
    import concourse.tile as tile
    from concourse.masks import make_identity

    f32 = mybir.dt.float32
    f16 = mybir.dt.float16
    f8 = mybir.dt.float8e4
    AF = mybir.ActivationFunctionType
    ALU = mybir.AluOpType

    nc = bacc.Bacc(
        "TRN2",
        target_bir_lowering=False,
        debug=False,
        enable_asserts=False,
        num_devices=N_CORES,
    )

    x_ap = nc.dram_tensor("x", (NS, D, L), f32, kind="ExternalInput").ap()
    w_ap = nc.dram_tensor("weight", (K, D), f32, kind="ExternalInput").ap()
    s_ap = nc.dram_tensor("scale", (1,), f32, kind="ExternalInput").ap()
    o_ap = nc.dram_tensor("out", (NS, K, L), f8, kind="ExternalOutput").ap()

    def ch(a, b):  # cols covering chunks [a, b)
        return slice(a * LC, b * LC)

    # Q0 (SWDGE cast-on-load) transfer plan: full images for the first
    # pair (their compute has slack), fine-grained interleaved pieces
    # for the second pair so each completion sem gates at most two
    # chunks of matmuls. Chunks 6-7 of n2/n3 arrive via HWDGE instead
    # (raw f32, cast on ACT) so the late tail has no SWDGE straggler
    # lag on its dependency sems.
    # Q0 transfer plan: graded sizes so PE starts by ~10us and is then
    # fed continuously (PE has ~14us of serial matmul work against an
    # ~18us stream — any idle gap lands on the tail). n2/n3 chunks 5-7
    # arrive via HWDGE (raw f32) instead of Q0, so the late pair-1 work
    # is ready mid-stream. 10 Q0 transfers keeps the SWDGE straggler
    # lag small (measured: ~0 at 8 transfers, 1.65us at 12).
    stream = [
        (0, ch(0, 1)), (1, ch(0, 1)),
        (0, ch(1, 4)), (1, ch(1, 4)),
        (0, ch(4, 8)), (1, ch(4, 8)),
        (2, ch(0, 3)), (3, ch(0, 3)),
        (2, ch(3, 5)), (3, ch(3, 5)),
    ]

    with tile.TileContext(nc) as tc:
        with (
            tc.tile_pool(name="consts", bufs=1) as consts,
            tc.tile_pool(name="xin", bufs=4) as xpool,
            tc.tile_pool(name="xsq", bufs=4) as xqpool,
            tc.tile_pool(name="outp", bufs=2) as opool,
            tc.tile_pool(name="psum", bufs=6, space="PSUM") as pspool,
            tc.tile_pool(name="psum1", bufs=1, space="PSUM") as pspool1,
        ):
            xts = [
                xpool.tile([D, L], f16, tag="xt", name=f"x_{n}")
                for n in range(NS)
            ]
            xqs = [
                xqpool.tile([D, L], f16, tag="xq", name=f"xsq_{n}")
                for n in range(NS)
            ]

            # ---- input stream (SWDGE Q0, cast f32->f16 on load) ----------
            ident = consts.tile([K, K], f32)
            for i, (n, sl) in enumerate(stream):
                nc.gpsimd.dma_start(out=xts[n][:, sl], in_=x_ap[n][:, sl])
                if i == 0:
                    make_identity(nc, ident)

            # ---- weight / scale prep (HWDGE, FIRST on the ring so they
            # land in ~1us — everything downstream needs wT16/bias2) ------
            s_t = consts.tile([1, 1], f32)
            nc.sync.dma_start(out=s_t, in_=s_ap.to_broadcast((1, 1)))
            w2 = consts.tile([2 * K, D], f32)
            nc.sync.dma_start(out=w2[0:K, :], in_=w_ap)
            nc.sync.dma_start(out=w2[K : 2 * K, :], in_=w_ap)

            # ---- HWDGE raw-f32 loads for chunks 5-7 of n2/n3 -------------
            # These drain alongside the Q0 stream and land mid-stream with
            # negligible completion lag. The f16 casts run on ACT (NOT
            # GpSimd: Pool tensor ops lock the shared DVE/GpSimd SBUF port
            # pair and knock DVE out of 2x perf mode); the squares run on
            # DVE straight from f32, so chunks 5-7 of the last pair are
            # compute-ready well before the Q0 stream ends.
            xfs = {}
            for n in (2, 3):
                xf = xpool.tile([D, 3 * LC], f32, tag="xf", name=f"xf_{n}")
                xfs[n] = xf
                nc.sync.dma_start(out=xf, in_=x_ap[n][:, ch(5, 8)])

            ones_row = consts.tile([1, 128], f32)
            nc.vector.memset(ones_row, 1.0)
            ones16 = consts.tile([D, K], f16)
            nc.vector.memset(ones16, 1.0)

            # broadcast scale to all 128 partitions via 1-col fp32 matmul
            ps_s = pspool1.tile([128, 1], f32, name="ps_s")
            nc.tensor.matmul(ps_s, ones_row, s_t, start=True, stop=True)
            s_b = consts.tile([128, 1], f32)
            nc.vector.tensor_scalar_mul(s_b, in0=ps_s, scalar1=1.0)
            inv_s2 = consts.tile([128, 1], f32)
            nc.vector.tensor_mul(inv_s2, s_b, s_b)
            nc.vector.reciprocal(inv_s2, inv_s2)

            w_sq = consts.tile([2 * K, D], f32)
            nc.vector.tensor_mul(w_sq, w2, w2)
            c_sq = consts.tile([2 * K, 1], f32)
            nc.vector.reduce_sum(out=c_sq, in_=w_sq, axis=mybir.AxisListType.X)
            c_sq_s = consts.tile([2 * K, 1], f32)
            nc.vector.tensor_mul(c_sq_s, c_sq, inv_s2)
            # fp8 offset encoding: store e4m3(dist - 2D/s^2); the host adds
            # the offset back. Centering kills the common mode so e4m3's
            # 6% relative step lands on the +-170 residual.
            bias2 = consts.tile([2 * K, 1], f32)
            nc.vector.tensor_scalar(
                out=bias2, in0=inv_s2,
                scalar1=-float(2 * D), scalar2=c_sq_s,
                op0=ALU.mult, op1=ALU.add,
            )

            ps_w = pspool1.tile([D, K], f32, name="ps_w")
            nc.tensor.transpose(ps_w, w2[0:K, :], ident)
            wT16 = consts.tile([D, K], f16)
            nc.vector.tensor_scalar_mul(wT16, in0=ps_w, scalar1=-2.0)
            # f32 copy of the stationary for the xf-fed chunks (their
            # moving operand stays raw f32 — no cast pass needed at all)
            wT32 = consts.tile([D, K], f32)
            nc.vector.tensor_scalar_mul(wT32, in0=ps_w, scalar1=-2.0)

            # ---- derived stream: fp16 x^2 on DVE, in arrival order -------
            # (the xf squares interleave between the n0/n1 bulk squares so
            # neither the pair-0 matmuls nor the early pair-1 matmuls
            # stall on DVE ordering)
            def sq(n, sl):
                nc.vector.tensor_mul(xqs[n][:, sl], xts[n][:, sl], xts[n][:, sl])

            for n, sl in stream[:5]:
                sq(n, sl)                                   # through n0 ch(4,8)
            nc.vector.tensor_mul(xqs[2][:, ch(5, 8)], xfs[2], xfs[2])
            sq(1, ch(4, 8))
            nc.vector.tensor_mul(xqs[3][:, ch(5, 8)], xfs[3], xfs[3])
            for n, sl in stream[6:]:
                sq(n, sl)

            # ---- matmuls + epilogues + HWDGE output pieces ---------------
            # Per-image DECOUPLED emission: a chunk's h0 col-group matmuls
            # (first image) run as soon as that image's piece lands; the
            # h64 group (second image) + epilogue follow when ITS piece
            # lands. PE executes in queue order, so the queue is laid out
            # in data-arrival order and PE never idles mid-stream.
            out_t0 = opool.tile([2 * K, L], f8, tag="out_t", name="out_0")
            out_t1 = opool.tile([2 * K, L], f8, tag="out_t", name="out_1")
            o_p0 = o_ap[0:2].rearrange("a k l -> (a k) l")
            o_p1 = o_ap[2:4].rearrange("a k l -> (a k) l")

            def mm_half(ps, img, c, half):
                sl = ch(c, c + 1)
                rows = slice(0, K) if half == 0 else slice(K, 2 * K)
                nc.tensor.matmul(
                    ps[rows, :], wT16, xts[img][:, sl],
                    start=True, stop=False, tile_position=(0, half),
                )
                nc.tensor.matmul(
                    ps[rows, :], ones16, xqs[img][:, sl],
                    start=False, stop=True, tile_position=(0, half),
                )

            def epi(out_t, ps, c, split=False):
                sl = ch(c, c + 1)
                if split:
                    nc.scalar.activation(
                        out_t[:, c * LC : c * LC + HC], ps[:, 0:HC],
                        AF.Identity, bias=bias2, scale=inv_s2,
                    )
                    nc.vector.tensor_scalar(
                        out=out_t[:, c * LC + HC : (c + 1) * LC],
                        in0=ps[:, HC:LC],
                        scalar1=inv_s2, scalar2=bias2,
                        op0=ALU.mult, op1=ALU.add,
                    )
                else:
                    nc.scalar.activation(
                        out_t[:, sl], ps, AF.Identity,
                        bias=bias2, scale=inv_s2,
                    )

            # pair 0: c0 both halves; then h0/h64 phases per Q0 piece
            pst = {}
            ps0 = pspool.tile([2 * K, LC], f32, name="ps")
            mm_half(ps0, 0, 0, 0)
            mm_half(ps0, 1, 0, 64)
            epi(out_t0, ps0, 0)
            for c in (1, 2, 3):
                pst[c] = pspool.tile([2 * K, LC], f32, name="ps")
                mm_half(pst[c], 0, c, 0)
            for c in (1, 2, 3):
                mm_half(pst[c], 1, c, 64)
                epi(out_t0, pst[c], c)
            for c in (4, 5, 6, 7):
                pst[c] = pspool.tile([2 * K, LC], f32, name="ps")
                mm_half(pst[c], 0, c, 0)
            for c in (4, 5, 6, 7):
                mm_half(pst[c], 1, c, 64)
                epi(out_t0, pst[c], c)
            nc.sync.dma_start(out=o_p0, in_=out_t0)

            # pair 1: HWDGE-fed chunks 5-7 first (ready mid-stream), with
            # fp32 W-matmuls straight off the raw xf tiles (no cast); then
            # the Q0-fed chunks in arrival order; the last-ready output
            # piece is the final 50 KB chunk ch(4,5).
            for c in (5, 6, 7):
                ps = pspool.tile([2 * K, LC], f32, name="ps")
                fsl = slice((c - 5) * LC, (c - 4) * LC)
                for img, half in ((2, 0), (3, 64)):
                    rows = slice(0, K) if half == 0 else slice(K, 2 * K)
                    nc.tensor.matmul(
                        ps[rows, :], wT32, xfs[img][:, fsl],
                        start=True, stop=False, tile_position=(0, half),
                    )
                    nc.tensor.matmul(
                        ps[rows, :], ones16, xqs[img][:, ch(c, c + 1)],
                        start=False, stop=True, tile_position=(0, half),
                    )
                epi(out_t1, ps, c)
            es = ch(5, 8)
            nc.sync.dma_start(out=o_p1[:, es], in_=out_t1[:, es])
            for c in (0, 1, 2):
                pst[c] = pspool.tile([2 * K, LC], f32, name="ps")
                mm_half(pst[c], 2, c, 0)
            for c in (0, 1, 2):
                mm_half(pst[c], 3, c, 64)
                epi(out_t1, pst[c], c)
            hs = ch(0, 3)
            nc.sync.dma_start(out=o_p1[:, hs], in_=out_t1[:, hs])
            for c in (3, 4):
                pst[c] = pspool.tile([2 * K, LC], f32, name="ps")
                mm_half(pst[c], 2, c, 0)
            for c in (3, 4):
                mm_half(pst[c], 3, c, 64)
                epi(out_t1, pst[c], c, split=True)
                piece = ch(c, c + 1)
                nc.sync.dma_start(
                    out=o_p1[:, piece], in_=out_t1[:, piece]
                )



    nc.compile()
    return nc


def _get_nc():
    if "nc" not in _cache:
        _cache["nc"] = _build()
    return _cache["nc"]


def run(x, weight, scale, trace=False, tmpdir=None):
    from concourse.bass_utils import run_bass_kernel_spmd

    x = np.ascontiguousarray(np.asarray(x, dtype=np.float32))
    weight = np.ascontiguousarray(np.asarray(weight, dtype=np.float32))
    scale = np.ascontiguousarray(np.asarray(scale, dtype=np.float32))
    assert x.shape == (N, D, L) and weight.shape == (K, D) and scale.shape == (1,)

    nc = _get_nc()
    in_maps = [
        {"x": x[c * NS : (c + 1) * NS], "weight": weight, "scale": scale}
        for c in range(N_CORES)
    ]
    res = run_bass_kernel_spmd(
        nc, in_maps, core_ids=list(range(N_CORES)), trace=trace, tmpdir=tmpdir
    )
    out = np.concatenate([r["out"] for r in res.results], axis=0).astype(np.float32)
    out += np.float32(2.0 * D) / np.float32(scale[0] ** 2)
    return out, res


def kernel(x, weight, scale):
    out, _ = run(x, weight, scale, trace=False)
    return out


# revision 37
# speedup vs baseline: 1.0313x; 1.0313x over previous
"""VQ codebook squared-distance kernel for Trainium2 (8 NeuronCores).

Computes dist[n,k,l] = (||x[n,:,l]||^2 + ||w[k,:]||^2 - 2*x[n,:,l].w[k,:]) / scale^2
for x (32,128,3136) f32, weight (64,128) f32, scale (1,) f32 -> out (32,64,3136) f32.

Sharding: data-parallel over N (4 per core); weight/scale replicated.

v3 design notes (per-core; v1 37.5us, v2 38.4us measured):
  - Input stream is HBM-stack-roofline-bound (~343 GB/s/core with both
    NCs of a stack active): 6.42 MB f32 x read in ~17.4us. Not
    improvable; everything else is about the edges of the stream.
  - v1's killer: one SDMA engine (the doc'd engine-7/15 SWDGE
    descriptor-ring port contention) lags the other 15, and the lag
    grows whenever the Q7 ring is being written. The last input
    transfer's completion sem fired 3.8us after its bytes landed,
    gating the whole tail. Mitigation here: coarse transfers (full
    images for n0..n2 -> 128 descriptors each instead of 256+) and
    NO output descriptors on the SWDGE ring at all.
  - Outputs ride HWDGE (nc.sync) instead: no Q7 ring writes, no
    queueing behind the straggler's input backlog, faster dispatch
    (RTL descgen ~0.6us vs Q7 ~1.65us). Output pieces are gated late
    (pair-0 ships as one full-L write when its last epilogue lands
    ~24us) to limit read/write turnaround mixing during the stream.
  - The last image (n3) streams in shrinking pieces (4/2/1/1 chunks)
    so the final dependency chain after the last input byte is just:
    2 matmuls + one 392-col epilogue (split ACT||DVE halves) + one
    50 KB HWDGE write + ~2us completion receipt.
  - Output is offset fp8: e4m3(dist - 2D/s^2), host adds the offset
    back; rel_l2 ~3e-3 vs the 2e-2 budget, half the write traffic.
  - PE: psum = (-2Wt)f16 @ x_f16 + ones_f16 @ (x^2)_f16, two images
    per PSUM tile via column tiling (tile_position (0,0)/(0,64)).
  - NEFF postamble: walrus emits a FIXED 257 per-semaphore clears
    split across the 5 engines (measured invariant to kernel size);
    the pacer is the Tensor sequencer at ~115ns/clear when the HAM
    clock gate has re-throttled (PE idle >3.4us). A dummy matmul
    gated on the final epilogue keeps PE at K=8/8 through most of
    the clear window (~57ns/clear), moving the pacer to Scalar.
  - scale broadcast 1->128 via 1-col fp32 matmul; weight transpose on
    PE (identity built early on gpsimd).
"""

import numpy as np

N, D, L, K = 32, 128, 3136, 64
N_CORES = 8
NS = N // N_CORES          # n's per core
LC = 392                   # matmul chunk (8 per image, one PSUM bank)
HC = 196                   # half-chunk for the split tail epilogues

_cache = {}


def _build():
    import concourse.bacc as bacc
    import concourse.mybir as mybir
    import concourse.tile as tile
    from concourse.masks import make_identity

    f32 = mybir.dt.float32
    f16 = mybir.dt.float16
    f8 = mybir.dt.float8e4
    AF = mybir.ActivationFunctionType
    ALU = mybir.AluOpType

    nc = bacc.Bacc(
        "TRN2",
        target_bir_lowering=False,
        debug=False,
        enable_asserts=False,
        num_devices=N_CORES,
    )

    x_ap = nc.dram_tensor("x", (NS, D, L), f32, kind="ExternalInput").ap()
    w_ap = nc.dram_tensor("weight", (K, D), f32, kind="ExternalInput").ap()
    s_ap = nc.dram_tensor("scale", (1,), f32, kind="ExternalInput").ap()
    o_ap = nc.dram_tensor("out", (NS, K, L), f8, kind="ExternalOutput").ap()

    def ch(a, b):  # cols covering chunks [a, b)
        return slice(a * LC, b * LC)

    # Q0 (SWDGE cast-on-load) transfer plan: full images for the first
    # pair (their compute has slack), fine-grained interleaved pieces
    # for the second pair so each completion sem gates at most two
    # chunks of matmuls. Chunks 6-7 of n2/n3 arrive via HWDGE instead
    # (raw f32, cast on ACT) so the late tail has no SWDGE straggler
    # lag on its dependency sems.
    # Q0 transfer plan: graded sizes so PE starts by ~10us and is then
    # fed continuously (PE has ~14us of serial matmul work against an
    # ~18us stream — any idle gap lands on the tail). n2/n3 chunks 5-7
    # arrive via HWDGE (raw f32) instead of Q0, so the late pair-1 work
    # is ready mid-stream. 10 Q0 transfers keeps the SWDGE straggler
    # lag small (measured: ~0 at 8 transfers, 1.65us at 12).
    stream = [
        (0, ch(0, 1)), (1, ch(0, 1)),
        (0, ch(1, 4)), (1, ch(1, 4)),
        (0, ch(4, 8)), (1, ch(4, 8)),
        (2, ch(0, 3)), (3, ch(0, 3)),
        (2, ch(3, 5)), (3, ch(3, 5)),
    ]

    with tile.TileContext(nc) as tc:
        with (
            tc.tile_pool(name="consts", bufs=1) as consts,
            tc.tile_pool(name="xin", bufs=4) as xpool,
            tc.tile_pool(name="xsq", bufs=4) as xqpool,
            tc.tile_pool(name="outp", bufs=2) as opool,
            tc.tile_pool(name="psum", bufs=6, space="PSUM") as pspool,
            tc.tile_pool(name="psum1", bufs=1, space="PSUM") as pspool1,
        ):
            xts = [
                xpool.tile([D, L], f16, tag="xt", name=f"x_{n}")
                for n in range(NS)
            ]
            xqs = [
                xqpool.tile([D, L], f16, tag="xq", name=f"xsq_{n}")
                for n in range(NS)
            ]

            # ---- input stream (SWDGE Q0, cast f32->f16 on load) ----------
            ident = consts.tile([K, K], f32)
            for i, (n, sl) in enumerate(stream):
                nc.gpsimd.dma_start(out=xts[n][:, sl], in_=x_ap[n][:, sl])
                if i == 0:
                    make_identity(nc, ident)

            # ---- weight / scale prep (HWDGE, FIRST on the ring so they
            # land in ~1us — everything downstream needs wT16/bias2) ------
            s_t = consts.tile([1, 1], f32)
            nc.sync.dma_start(out=s_t, in_=s_ap.to_broadcast((1, 1)))
            w2 = consts.tile([2 * K, D], f32)
            nc.sync.dma_start(out=w2[0:K, :], in_=w_ap)
            nc.sync.dma_start(out=w2[K : 2 * K, :], in_=w_ap)

            # ---- HWDGE raw-f32 loads for chunks 5-7 of n2/n3 -------------
            # These drain alongside the Q0 stream and land mid-stream with
            # negligible completion lag. The f16 casts run on ACT (NOT
            # GpSimd: Pool tensor ops lock the shared DVE/GpSimd SBUF port
            # pair and knock DVE out of 2x perf mode); the squares run on
            # DVE straight from f32, so chunks 5-7 of the last pair are
            # compute-ready well before the Q0 stream ends.
            xfs = {}
            for n in (2, 3):
                xf = xpool.tile([D, 3 * LC], f32, tag="xf", name=f"xf_{n}")
                xfs[n] = xf
                nc.sync.dma_start(out=xf, in_=x_ap[n][:, ch(5, 8)])

            ones_row = consts.tile([1, 128], f32)
            nc.vector.memset(ones_row, 1.0)
            ones16 = consts.tile([D, K], f16)
            nc.vector.memset(ones16, 1.0)

            # broadcast scale to all 128 partitions via 1-col fp32 matmul
            ps_s = pspool1.tile([128, 1], f32, name="ps_s")
            nc.tensor.matmul(ps_s, ones_row, s_t, start=True, stop=True)
            s_b = consts.tile([128, 1], f32)
            nc.vector.tensor_scalar_mul(s_b, in0=ps_s, scalar1=1.0)
            inv_s2 = consts.tile([128, 1], f32)
            nc.vector.tensor_mul(inv_s2, s_b, s_b)
            nc.vector.reciprocal(inv_s2, inv_s2)

            w_sq = consts.tile([2 * K, D], f32)
            nc.vector.tensor_mul(w_sq, w2, w2)
            c_sq = consts.tile([2 * K, 1], f32)
            nc.vector.reduce_sum(out=c_sq, in_=w_sq, axis=mybir.AxisListType.X)
            c_sq_s = consts.tile([2 * K, 1], f32)
            nc.vector.tensor_mul(c_sq_s, c_sq, inv_s2)
            # fp8 offset encoding: store e4m3(dist - 2D/s^2); the host adds
            # the offset back. Centering kills the common mode so e4m3's
            # 6% relative step lands on the +-170 residual.
            bias2 = consts.tile([2 * K, 1], f32)
            nc.vector.tensor_scalar(
                out=bias2, in0=inv_s2,
                scalar1=-float(2 * D), scalar2=c_sq_s,
                op0=ALU.mult, op1=ALU.add,
            )

            ps_w = pspool1.tile([D, K], f32, name="ps_w")
            nc.tensor.transpose(ps_w, w2[0:K, :], ident)
            wT16 = consts.tile([D, K], f16)
            nc.vector.tensor_scalar_mul(wT16, in0=ps_w, scalar1=-2.0)

            # ---- derived stream: fp16 x^2 on DVE, in arrival order -------
            # (the xf squares interleave between the n0/n1 bulk squares so
            # neither the pair-0 matmuls nor the early pair-1 matmuls
            # stall on DVE ordering)
            def sq(n, sl):
                nc.vector.tensor_mul(xqs[n][:, sl], xts[n][:, sl], xts[n][:, sl])

            for n, sl in stream[:5]:
                sq(n, sl)                                   # through n0 ch(4,8)
            nc.vector.tensor_mul(xqs[2][:, ch(5, 8)], xfs[2], xfs[2])
            sq(1, ch(4, 8))
            nc.vector.tensor_mul(xqs[3][:, ch(5, 8)], xfs[3], xfs[3])
            for n, sl in stream[6:]:
                sq(n, sl)

            # ---- matmuls + epilogues + HWDGE output pieces ---------------
            # Per-image DECOUPLED emission: a chunk's h0 col-group matmuls
            # (first image) run as soon as that image's piece lands; the
            # h64 group (second image) + epilogue follow when ITS piece
            # lands. PE executes in queue order, so the queue is laid out
            # in data-arrival order and PE never idles mid-stream.
            out_t0 = opool.tile([2 * K, L], f8, tag="out_t", name="out_0")
            out_t1 = opool.tile([2 * K, L], f8, tag="out_t", name="out_1")
            o_p0 = o_ap[0:2].rearrange("a k l -> (a k) l")
            o_p1 = o_ap[2:4].rearrange("a k l -> (a k) l")

            def mm_half(ps, img, c, half):
                sl = ch(c, c + 1)
                rows = slice(0, K) if half == 0 else slice(K, 2 * K)
                nc.tensor.matmul(
                    ps[rows, :], wT16, xts[img][:, sl],
                    start=True, stop=False, tile_position=(0, half),
                )
                nc.tensor.matmul(
                    ps[rows, :], ones16, xqs[img][:, sl],
                    start=False, stop=True, tile_position=(0, half),
                )

            def epi(out_t, ps, c, split=False):
                sl = ch(c, c + 1)
                if split:
                    nc.scalar.activation(
                        out_t[:, c * LC : c * LC + HC], ps[:, 0:HC],
                        AF.Identity, bias=bias2, scale=inv_s2,
                    )
                    nc.vector.tensor_scalar(
                        out=out_t[:, c * LC + HC : (c + 1) * LC],
                        in0=ps[:, HC:LC],
                        scalar1=inv_s2, scalar2=bias2,
                        op0=ALU.mult, op1=ALU.add,
                    )
                else:
                    nc.scalar.activation(
                        out_t[:, sl], ps, AF.Identity,
                        bias=bias2, scale=inv_s2,
                    )

            # pair 0: c0 both halves; then h0/h64 phases per Q0 piece
            pst = {}
            ps0 = pspool.tile([2 * K, LC], f32, name="ps")
            mm_half(ps0, 0, 0, 0)
            mm_half(ps0, 1, 0, 64)
            epi(out_t0, ps0, 0)
            for c in (1, 2, 3):
                pst[c] = pspool.tile([2 * K, LC], f32, name="ps")
                mm_half(pst[c], 0, c, 0)
            for c in (1, 2, 3):
                mm_half(pst[c], 1, c, 64)
                epi(out_t0, pst[c], c)
            # xf -> f16 casts, placed here in ACT program order: after the
            # phase-A epilogues (so they don't block them) and before the
            # phase-B ones (so the casts are done by the time pair 1's
            # chunks 5-7 need them)
            for n in (2, 3):
                nc.scalar.activation(xts[n][:, ch(5, 8)], xfs[n], AF.Identity)
            for c in (4, 5, 6, 7):
                pst[c] = pspool.tile([2 * K, LC], f32, name="ps")
                mm_half(pst[c], 0, c, 0)
            for c in (4, 5, 6, 7):
                mm_half(pst[c], 1, c, 64)
                epi(out_t0, pst[c], c)
            nc.sync.dma_start(out=o_p0, in_=out_t0)

            # pair 1: HWDGE-fed chunks 5-7 first (ready mid-stream), then
            # the Q0-fed chunks in arrival order; the last-ready output
            # piece is the final 50 KB chunk ch(4,5).
            for c in (5, 6, 7):
                ps = pspool.tile([2 * K, LC], f32, name="ps")
                mm_half(ps, 2, c, 0)
                mm_half(ps, 3, c, 64)
                epi(out_t1, ps, c)
            es = ch(5, 8)
            nc.sync.dma_start(out=o_p1[:, es], in_=out_t1[:, es])
            for c in (0, 1, 2):
                pst[c] = pspool.tile([2 * K, LC], f32, name="ps")
                mm_half(pst[c], 2, c, 0)
            for c in (0, 1, 2):
                mm_half(pst[c], 3, c, 64)
                epi(out_t1, pst[c], c, split=True)
            hs = ch(0, 3)
            nc.sync.dma_start(out=o_p1[:, hs], in_=out_t1[:, hs])
            for c in (3, 4):
                pst[c] = pspool.tile([2 * K, LC], f32, name="ps")
                mm_half(pst[c], 2, c, 0)
            for c in (3, 4):
                mm_half(pst[c], 3, c, 64)
                epi(out_t1, pst[c], c, split=True)
                piece = ch(c, c + 1)
                nc.sync.dma_start(
                    out=o_p1[:, piece], in_=out_t1[:, piece]
                )



    nc.compile()
    return nc


def _get_nc():
    if "nc" not in _cache:
        _cache["nc"] = _build()
    return _cache["nc"]


def run(x, weight, scale, trace=False, tmpdir=None):
    from concourse.bass_utils import run_bass_kernel_spmd

    x = np.ascontiguousarray(np.asarray(x, dtype=np.float32))
    weight = np.ascontiguousarray(np.asarray(weight, dtype=np.float32))
    scale = np.ascontiguousarray(np.asarray(scale, dtype=np.float32))
    assert x.shape == (N, D, L) and weight.shape == (K, D) and scale.shape == (1,)

    nc = _get_nc()
    in_maps = [
        {"x": x[c * NS : (c + 1) * NS], "weight": weight, "scale": scale}
        for c in range(N_CORES)
    ]
    res = run_bass_kernel_spmd(
        nc, in_maps, core_ids=list(range(N_CORES)), trace=trace, tmpdir=tmpdir
    )
    out = np.concatenate([r["out"] for r in res.results], axis=0).astype(np.float32)
    out += np.float32(2.0 * D) / np.float32(scale[0] ** 2)
    return out, res


def kernel(x, weight, scale):
    out, _ = run(x, weight, scale, trace=False)
    return out


# revision 38
# speedup vs baseline: 1.0581x; 1.0260x over previous
"""VQ codebook squared-distance kernel for Trainium2 (8 NeuronCores).

Computes dist[n,k,l] = (||x[n,:,l]||^2 + ||w[k,:]||^2 - 2*x[n,:,l].w[k,:]) / scale^2
for x (32,128,3136) f32, weight (64,128) f32, scale (1,) f32 -> out (32,64,3136) f32.

Sharding: data-parallel over N (4 per core); weight/scale replicated.

Design (best-measured configuration, 37.04us; nine HW iterations):
  - Input stream is HBM-stack-roofline-bound (~343 GB/s/core with both
    NCs of a stack active): 6.42 MB f32 x read in ~18us. The kernel is
    a saturated multi-resource equilibrium: PE ~14-16us, ACT ~14us,
    DVE ~13us busy inside a ~22us window, plus a FIXED ~8.7us NEFF
    end block (barrier + 257 walrus per-semaphore clears, invariant
    to kernel content and not HAM-gated).
  - x loads via SWDGE Q0 cast-on-load f32->f16 in graded pieces:
    small head (PE starts by ~11us), fine interleaved tail so each
    completion sem gates at most two chunks of matmuls. NOTE the
    SWDGE straggler: one SDMA engine lags the other 15 by an amount
    that grows with Q0 descriptor pressure (~0 at 8 transfers,
    ~1.65us at 12, ~2.4-4us beyond or with 3+ full-image transfers).
  - Chunks 6-7 of n2/n3 arrive via HWDGE (raw f32, ACT-cast to f16,
    DVE-squared straight from f32): lag-free sems, ready mid-stream.
    Never put the casts on GpSimd - Pool tensor ops lock the shared
    DVE/GpSimd SBUF ports and knock DVE out of 2x perf mode.
  - Outputs ride HWDGE in readiness-ordered pieces (no Q0 descriptor
    traffic, no queueing behind the straggler): pair 0 as one full-L
    write, pair 1 as ch(6,8)/ch(0,3)/ch(3,5)/ch(5,6) with the final
    50 KB piece last; late epilogues split ACT || DVE.
  - Output is offset fp8: e4m3(dist - 2D/s^2), host adds the offset
    back. Centering removes the ~2D/s^2 common mode so e4m3's ~6%
    step applies to the +-170 residual only: rel_l2 ~3e-3 (vs 2e-2
    budget) for half the write traffic.
  - PE: psum = (-2Wt)f16 @ x_f16 + ones_f16 @ (x^2)_f16, two images
    per PSUM tile via column tiling (tile_position (0,0)/(0,64)).
  - scale broadcast 1->128 via 1-col fp32 matmul; weight transpose on
    PE (identity built early on gpsimd).
"""

import numpy as np

N, D, L, K = 32, 128, 3136, 64
N_CORES = 8
NS = N // N_CORES          # n's per core
LC = 392                   # matmul chunk (8 per image, one PSUM bank)
HC = 196                   # half-chunk for the split tail epilogues

_cache = {}


def _build():
    import concourse.bacc as bacc
    import concourse.mybir as mybir
    import concourse.tile as tile
    from concourse.masks import make_identity

    f32 = mybir.dt.float32
    f16 = mybir.dt.float16
    f8 = mybir.dt.float8e4
    AF = mybir.ActivationFunctionType
    ALU = mybir.AluOpType

    nc = bacc.Bacc(
        "TRN2",
        target_bir_lowering=False,
        debug=False,
        enable_asserts=False,
        num_devices=N_CORES,
    )

    x_ap = nc.dram_tensor("x", (NS, D, L), f32, kind="ExternalInput").ap()
    w_ap = nc.dram_tensor("weight", (K, D), f32, kind="ExternalInput").ap()
    s_ap = nc.dram_tensor("scale", (1,), f32, kind="ExternalInput").ap()
    o_ap = nc.dram_tensor("out", (NS, K, L), f8, kind="ExternalOutput").ap()

    def ch(a, b):  # cols covering chunks [a, b)
        return slice(a * LC, b * LC)

    # Q0 (SWDGE cast-on-load) transfer plan: graded sizes so PE starts
    # early and is then fed continuously; fine-grained interleaved
    # pieces for the second pair so each completion sem gates at most
    # two chunks of matmuls.
    stream = [
        (0, ch(0, 2)), (1, ch(0, 2)),
        (0, ch(2, 5)), (1, ch(2, 5)),
        (0, ch(5, 8)), (1, ch(5, 8)),
        (2, ch(0, 3)), (3, ch(0, 3)),
        (2, ch(3, 5)), (3, ch(3, 5)),
        (2, ch(5, 6)), (3, ch(5, 6)),
    ]

    with tile.TileContext(nc) as tc:
        with (
            tc.tile_pool(name="consts", bufs=1) as consts,
            tc.tile_pool(name="xin", bufs=4) as xpool,
            tc.tile_pool(name="xsq", bufs=4) as xqpool,
            tc.tile_pool(name="outp", bufs=2) as opool,
            tc.tile_pool(name="psum", bufs=4, space="PSUM") as pspool,
            tc.tile_pool(name="psum1", bufs=1, space="PSUM") as pspool1,
        ):
            xts = [
                xpool.tile([D, L], f16, tag="xt", name=f"x_{n}")
                for n in range(NS)
            ]
            xqs = [
                xqpool.tile([D, L], f16, tag="xq", name=f"xsq_{n}")
                for n in range(NS)
            ]

            # ---- input stream (SWDGE Q0, cast f32->f16 on load) ----------
            ident = consts.tile([K, K], f32)
            for i, (n, sl) in enumerate(stream):
                nc.gpsimd.dma_start(out=xts[n][:, sl], in_=x_ap[n][:, sl])
                if i == 0:
                    make_identity(nc, ident)

            # ---- HWDGE raw-f32 loads for the tail chunks of n2/n3 --------
            # These drain alongside the Q0 stream and land mid-stream with
            # negligible completion lag; ACT casts them to f16 and DVE
            # squares them straight from f32, so chunks 6-7 of the last
            # pair are fully compute-ready before the Q0 stream even ends.
            xfs = {}
            for n in (2, 3):
                xf = xpool.tile([D, 2 * LC], f32, tag="xf", name=f"xf_{n}")
                xfs[n] = xf
                nc.sync.dma_start(out=xf, in_=x_ap[n][:, ch(6, 8)])
            for n in (2, 3):
                nc.scalar.activation(
                    xts[n][:, ch(6, 8)], xfs[n], AF.Identity,
                )

            # ---- weight / scale prep (HWDGE, overlaps the stream) --------
            s_t = consts.tile([1, 1], f32)
            nc.sync.dma_start(out=s_t, in_=s_ap.to_broadcast((1, 1)))
            w2 = consts.tile([2 * K, D], f32)
            nc.sync.dma_start(out=w2[0:K, :], in_=w_ap)
            nc.sync.dma_start(out=w2[K : 2 * K, :], in_=w_ap)

            ones_row = consts.tile([1, 128], f32)
            nc.vector.memset(ones_row, 1.0)
            ones16 = consts.tile([D, K], f16)
            nc.vector.memset(ones16, 1.0)

            # broadcast scale to all 128 partitions via 1-col fp32 matmul
            ps_s = pspool1.tile([128, 1], f32, name="ps_s")
            nc.tensor.matmul(ps_s, ones_row, s_t, start=True, stop=True)
            s_b = consts.tile([128, 1], f32)
            nc.vector.tensor_scalar_mul(s_b, in0=ps_s, scalar1=1.0)
            inv_s2 = consts.tile([128, 1], f32)
            nc.vector.tensor_mul(inv_s2, s_b, s_b)
            nc.vector.reciprocal(inv_s2, inv_s2)

            w_sq = consts.tile([2 * K, D], f32)
            nc.vector.tensor_mul(w_sq, w2, w2)
            c_sq = consts.tile([2 * K, 1], f32)
            nc.vector.reduce_sum(out=c_sq, in_=w_sq, axis=mybir.AxisListType.X)
            c_sq_s = consts.tile([2 * K, 1], f32)
            nc.vector.tensor_mul(c_sq_s, c_sq, inv_s2)
            # fp8 offset encoding: store e4m3(dist - 2D/s^2); the host adds
            # the offset back. Centering kills the common mode so e4m3's
            # 6% relative step lands on the +-170 residual.
            bias2 = consts.tile([2 * K, 1], f32)
            nc.vector.tensor_scalar(
                out=bias2, in0=inv_s2,
                scalar1=-float(2 * D), scalar2=c_sq_s,
                op0=ALU.mult, op1=ALU.add,
            )

            ps_w = pspool1.tile([D, K], f32, name="ps_w")
            nc.tensor.transpose(ps_w, w2[0:K, :], ident)
            wT16 = consts.tile([D, K], f16)
            nc.vector.tensor_scalar_mul(wT16, in0=ps_w, scalar1=-2.0)

            # ---- derived stream: fp16 x^2 on DVE, in arrival order -------
            # (the HWDGE-loaded tail chunks are squared straight from f32,
            # ordered after the n0/n1 squares so DVE never stalls on them)
            for n, sl in stream[:6]:
                nc.vector.tensor_mul(xqs[n][:, sl], xts[n][:, sl], xts[n][:, sl])
            for n in (2, 3):
                nc.vector.tensor_mul(xqs[n][:, ch(6, 8)], xfs[n], xfs[n])
            for n, sl in stream[6:]:
                nc.vector.tensor_mul(xqs[n][:, sl], xts[n][:, sl], xts[n][:, sl])

            # ---- matmuls + epilogues + HWDGE output pieces ---------------
            for pair in range(NS // 2):
                n0, n1 = 2 * pair, 2 * pair + 1
                out_t = opool.tile([2 * K, L], f8, tag="out_t", name=f"out_{pair}")
                o_pair = o_ap[2 * pair : 2 * pair + 2].rearrange("a k l -> (a k) l")
                last_pair = pair == NS // 2 - 1
                # pair 1's chunks are emitted in data-arrival order: the
                # HWDGE-fed chunks 6-7 are ready mid-stream, well before
                # the Q0-gated chunks; PE executes its queue in order.
                chunk_order = [6, 7, 0, 1, 2, 3, 4, 5] if last_pair else range(8)
                for c in chunk_order:
                    sl = ch(c, c + 1)
                    ps = pspool.tile([2 * K, LC], f32, name="ps")
                    nc.tensor.matmul(
                        ps[0:K, :], wT16, xts[n0][:, sl],
                        start=True, stop=False, tile_position=(0, 0),
                    )
                    nc.tensor.matmul(
                        ps[K : 2 * K, :], wT16, xts[n1][:, sl],
                        start=True, stop=False, tile_position=(0, 64),
                    )
                    nc.tensor.matmul(
                        ps[0:K, :], ones16, xqs[n0][:, sl],
                        start=False, stop=True, tile_position=(0, 0),
                    )
                    nc.tensor.matmul(
                        ps[K : 2 * K, :], ones16, xqs[n1][:, sl],
                        start=False, stop=True, tile_position=(0, 64),
                    )
                    if last_pair and c in (4, 5):
                        # split the late epilogues ACT || DVE so each clears
                        # in ~0.35us instead of ~0.7us (and ACT's backlog
                        # doesn't stack onto the final chunk)
                        nc.scalar.activation(
                            out_t[:, c * LC : c * LC + HC],
                            ps[:, 0:HC], AF.Identity,
                            bias=bias2, scale=inv_s2,
                        )
                        nc.vector.tensor_scalar(
                            out=out_t[:, c * LC + HC : (c + 1) * LC],
                            in0=ps[:, HC:LC],
                            scalar1=inv_s2, scalar2=bias2,
                            op0=ALU.mult, op1=ALU.add,
                        )
                    else:
                        nc.scalar.activation(
                            out_t[:, sl], ps, AF.Identity,
                            bias=bias2, scale=inv_s2,
                        )
                    # ship finished columns on HWDGE: pair 0 as one full-L
                    # write; pair 1 in pieces ordered by readiness, so the
                    # last-ready piece is the final 50 KB chunk ch(5,6).
                    if not last_pair:
                        if c == 7:
                            nc.sync.dma_start(out=o_pair, in_=out_t)
                    elif c == 7:
                        es = ch(6, 8)
                        nc.sync.dma_start(out=o_pair[:, es], in_=out_t[:, es])
                    elif c == 2:
                        hs = ch(0, 3)
                        nc.sync.dma_start(out=o_pair[:, hs], in_=out_t[:, hs])
                    elif c == 4:
                        qs = ch(3, 5)
                        nc.sync.dma_start(out=o_pair[:, qs], in_=out_t[:, qs])
                    elif c == 5:
                        fs = ch(5, 6)
                        nc.sync.dma_start(out=o_pair[:, fs], in_=out_t[:, fs])

    nc.compile()
    return nc


def _get_nc():
    if "nc" not in _cache:
        _cache["nc"] = _build()
    return _cache["nc"]


def run(x, weight, scale, trace=False, tmpdir=None):
    from concourse.bass_utils import run_bass_kernel_spmd

    x = np.ascontiguousarray(np.asarray(x, dtype=np.float32))
    weight = np.ascontiguousarray(np.asarray(weight, dtype=np.float32))
    scale = np.ascontiguousarray(np.asarray(scale, dtype=np.float32))
    assert x.shape == (N, D, L) and weight.shape == (K, D) and scale.shape == (1,)

    nc = _get_nc()
    in_maps = [
        {"x": x[c * NS : (c + 1) * NS], "weight": weight, "scale": scale}
        for c in range(N_CORES)
    ]
    res = run_bass_kernel_spmd(
        nc, in_maps, core_ids=list(range(N_CORES)), trace=trace, tmpdir=tmpdir
    )
    out = np.concatenate([r["out"] for r in res.results], axis=0).astype(np.float32)
    out += np.float32(2.0 * D) / np.float32(scale[0] ** 2)
    return out, res


def kernel(x, weight, scale):
    out, _ = run(x, weight, scale, trace=False)
    return out
